# revision 12
# baseline (speedup 1.0000x reference)
"""Trainium2 Bass kernel for the intra-batch point-cloud contrastive loss.

Math (matches the reference exactly):
  feats   = features_in.reshape(C, M).T    (row-major reinterpret), M = B*N
  labels  = labels_in.reshape(-1)
  sel     = bernoulli(key 42, min(750/(count+1),1)[labels])   (host, jax CPU)
  nv      = feats / ||feats||
  dp      = exp(nv @ nv.T / TEMP), diagonal zeroed
  pos_i   = sum_{j sel, same class} dp_ij ; neg over different class
  loss    = mean over selected i of -log(pos/(pos+neg))

Strategy (triangle/circulant, exp-minimal):
  Only selected points matter (~37% of M).  The selected points are
  class-sorted and each class padded to a 128-col boundary, so every
  128-block of columns is class-pure.  dp is symmetric, so each
  unordered block-pair is computed ONCE: row-block r covers col-blocks
  c = r+d (mod NB) for d in 0..NB/2.  Each core gets NB/8 row-blocks
  (circulant: core k takes global blocks {k, k+8, ...}; its input is
  rolled by k*128 so all slice offsets are SPMD-uniform).

  Per row-block window [128, W*128] (W = NB/2+1 blocks):
    PE  : sim chunks = lhsT.T @ nv  (bf16, K=66: 64 feature rows + a
          "-1e9 * pad" row pair masking pad rows/cols pre-exp), plus an
          eye@negeye matmul adding -1e9 on the d=0 diagonal
    ACT : dp = exp(sim/TEMP) -> bf16 SBUF, one instruction per window
          (exp volume is the throughput wall; the triangle halves it)
    DVE : per-128-block row sums (one 3D-AP tensor_reduce per window)
          -> class-pure partials, class-mapped on the host
    Pool: column sums of blocks d=1..W-2 via partition_all_reduce (the
          transpose-side contribution; d=0/W-1 excluded to avoid double
          counting)
  The host maps partials to classes and runs the O(n_sel) epilogue.
  Instruction count is kept minimal (~30 PE instrs): the fixed
  ~57-semaphore teardown sweep runs at ~280 ns/op on the PE sequencer,
  so a long PE instruction stream directly extends exec time.
"""

import numpy as np

TEMP = 0.07
NUM_CLASSES = 4
N_CORES = 8
P = 128

_NEFF_CACHE = {}


def _compute_sel(labels_flat):
    """Selection mask, bit-exact with the reference (jax threefry, key 42)."""
    import jax
    import jax.numpy as jnp

    cpu = jax.devices("cpu")[0]
    with jax.default_device(cpu):
        lab_j = jnp.asarray(labels_flat)
        counts = jnp.bincount(lab_j, length=NUM_CLASSES)
        keep_p = jnp.minimum(750.0 / (counts.astype(jnp.float32) + 1.0), 1.0)
        p = keep_p[lab_j]
        sel = jax.random.bernoulli(jax.random.key(42), p)
        return np.asarray(sel)


def _build_kernel(NBp):
    import concourse.bass as bass
    import concourse.bass_isa as bass_isa
    import concourse.mybir as mybir
    import concourse.tile as tile

    Mp = NBp * P
    rpc = NBp // N_CORES                  # row-blocks per core
    W = NBp // 2 + 1                      # window width (blocks)
    WP = W * P
    CSW = (W - 2) * P                     # colsum width per row-block
    f32 = mybir.dt.float32
    bf16 = mybir.dt.bfloat16
    K = 66                                # 64 features + colmask + rowmask

    nc = bass.Bass()
    nv_d = nc.dram_tensor("nv", [K, Mp], bf16, kind="ExternalInput")
    # colhs: eye [0:128], negeye-wide [128:640], ones col [640:641],
    # lhs blocks [641 : 641+rpc*128] (partitions 0..65)
    colhs_d = nc.dram_tensor("colhs", [P, 641 + rpc * P], bf16,
                             kind="ExternalInput")
    rs_d = nc.dram_tensor("rs_out", [P, rpc * W], f32, kind="ExternalOutput")
    cs_d = nc.dram_tensor("cs_out", [1, rpc * CSW], f32, kind="ExternalOutput")

    with tile.TileContext(nc) as tc:
        with (
            tc.tile_pool(name="singles", bufs=1) as singles,
            tc.tile_pool(name="dp", bufs=2) as dp_pool,
            tc.tile_pool(name="ps", bufs=2, space="PSUM") as ps_pool,
        ):
            colhs_sb = singles.tile([P, 641 + rpc * P], bf16)
            nv_sb = singles.tile([K, Mp], bf16)
            nc.gpsimd.dma_start(out=colhs_sb[:], in_=colhs_d[:])
            # first split covers row-block 0's whole window
            cut = min(WP, Mp)
            nc.gpsimd.dma_start(out=nv_sb[:, 0:cut], in_=nv_d[:, 0:cut])
            if cut < Mp:
                nc.gpsimd.dma_start(out=nv_sb[:, cut:Mp], in_=nv_d[:, cut:Mp])

            eye = colhs_sb[:, 0:P]
            negw = colhs_sb[:, P:P + 512]
            lhs_sb = colhs_sb[0:K, 641:641 + rpc * P]

            rs_sb = singles.tile([P, rpc * W], f32)
            cs_sb = singles.tile([1, rpc * CSW], f32)

            for b in range(rpc):
                lhs_b = lhs_sb[:, b * P:(b + 1) * P]
                ps = ps_pool.tile([P, WP], f32, name="ps")
                off = 0
                while off < WP:
                    w512 = min(512, WP - off)
                    # rolled col of this piece (wraps only at 1024 bounds)
                    src = (b * 1024 + off) % Mp
                    nc.tensor.matmul(
                        ps[:, off:off + w512],
                        lhs_b,
                        nv_sb[:, src:src + w512],
                        start=True,
                        stop=off != 0,
                    )
                    if off == 0:
                        # adds -1e9 on the diagonal of window block 0
                        nc.tensor.matmul(
                            ps[:, 0:w512], eye, negw[:, 0:w512],
                            start=False, stop=True,
                        )
                    off += w512
                dp = dp_pool.tile([P, WP], bf16)
                nc.scalar.activation(
                    dp[:], ps[:], mybir.ActivationFunctionType.Exp,
                    scale=float(1.0 / TEMP),
                )
                # per-block row sums (class-pure partials)
                nc.vector.tensor_reduce(
                    rs_sb[:, b * W:(b + 1) * W],
                    dp[:].rearrange("p (a b) -> p a b", b=P),
                    mybir.AxisListType.X,
                    mybir.AluOpType.add,
                )
                # column sums of window blocks 1..W-2
                nc.gpsimd.tensor_reduce(
                    cs_sb[0:1, b * CSW:(b + 1) * CSW],
                    dp[:, P:P + CSW],
                    mybir.AxisListType.C,
                    mybir.AluOpType.add,
                )
            nc.gpsimd.dma_start(out=cs_d[:], in_=cs_sb[0:1, :])
            nc.gpsimd.dma_start(out=rs_d[:], in_=rs_sb[:])

    _split_multi_waits(nc)
    return nc


def _split_multi_waits(nc):
    """Walrus in this toolchain accepts only one inline sync-wait per
    instruction.  Tile's kernel-tail drain aggregates one wait per live
    semaphore, so hoist all but the last wait onto nops.

    Drain waits go to SP-sequencer nops: the PE sequencer retires a
    semaphore op in ~280 ns vs ~25 ns on SP, so leaving the aggregated
    end-of-kernel waits on PE adds ~5 us of pure teardown.  Mid-kernel
    (non-drain) waits must stay on the owning engine to preserve
    ordering."""
    import concourse.mybir as mybir

    for fn in nc.m.functions:
        for blk in fn.blocks:
            insts = list(blk.instructions)
            out = []
            for inst in insts:
                si = inst.sync_info
                waits = list(si.on_wait) if si is not None and si.on_wait else []
                is_drain = isinstance(inst, mybir.InstDrain)
                if is_drain and waits:
                    for w in waits:
                        out.append(mybir.InstNoOp(
                            name=nc.get_next_instruction_name(),
                            engine=mybir.EngineType.SP,
                            bass_nofuse=True,
                            sync_info=mybir.SyncInfo(on_wait=[w], on_update=[]),
                        ))
                    si.on_wait = []
                elif len(waits) > 1:
                    for w in waits[:-1]:
                        out.append(mybir.InstNoOp(
                            name=nc.get_next_instruction_name(),
                            engine=inst.engine,
                            bass_nofuse=True,
                            sync_info=mybir.SyncInfo(on_wait=[w], on_update=[]),
                        ))
                    si.on_wait = waits[-1:]
                out.append(inst)
            if len(out) != len(insts):
                blk.instructions = out
    return nc


def _get_kernel(NBp):
    if NBp not in _NEFF_CACHE:
        _NEFF_CACHE[NBp] = _build_kernel(NBp)
    return _NEFF_CACHE[NBp]


def kernel(features_in, labels_in, _trace=False, _results=[None]):
    import ml_dtypes
    from concourse.bass_utils import run_bass_kernel_spmd

    bf16 = ml_dtypes.bfloat16
    features_in = np.asarray(features_in, dtype=np.float32)
    B, C, N = features_in.shape
    M = B * N
    labels = np.asarray(labels_in).reshape(-1).astype(np.int64)

    fT = features_in.reshape(C, M)                      # [C, M] reinterpret
    sel = _compute_sel(labels)
    idx = np.nonzero(sel)[0]
    lab_sel = labels[idx]
    n_sel = int(idx.size)
    n_div = max(n_sel, 1)

    # class-sorted, per-class 128-padded column layout
    order = np.argsort(lab_sel, kind="stable")
    idx_sorted = idx[order]
    lab_sorted = lab_sel[order]
    cnt = np.bincount(lab_sel, minlength=NUM_CLASSES)
    cls_blocks = np.maximum(np.ceil(cnt / P).astype(int), cnt > 0)
    NB = max(int(cls_blocks.sum()), 1)
    NBp = max(8 * int(np.ceil(NB / 8)), 8)
    Mp = NBp * P
    rpc = NBp // N_CORES
    W = NBp // 2 + 1
    CSW = (W - 2) * P

    norms = np.sqrt(np.sum(fT * fT, axis=0, dtype=np.float32))
    nvT = (fT / norms).astype(np.float32)

    col_of_point = np.zeros(n_sel, np.int64)
    block_class = np.full(NBp, -1, np.int64)
    nv = np.zeros((C, Mp), np.float32)
    padcol = np.ones(Mp, bool)
    b0 = 0
    pos_pt = 0
    for c in range(NUM_CLASSES):
        start = b0 * P
        n = int(cnt[c])
        sl = slice(pos_pt, pos_pt + n)
        col_of_point[sl] = start + np.arange(n)
        nv[:, start:start + n] = nvT[:, idx_sorted[sl]]
        padcol[start:start + n] = False
        block_class[b0:b0 + int(cls_blocks[c])] = c
        b0 += int(cls_blocks[c])
        pos_pt += n

    K = 66
    nv_ext = np.zeros((K, Mp), np.float32)
    nv_ext[:C] = nv
    nv_ext[C] = -1e9 * padcol                 # colmask row
    nv_ext[C + 1] = 1.0                       # pairs with lhs rowmask row

    colhs = np.zeros((P, 641 + rpc * P), np.float32)
    colhs[:, 0:P] = np.eye(P, dtype=np.float32)
    colhs[np.arange(P), P + np.arange(P)] = -1e9   # negeye (wide, zero-padded)
    colhs[:, 640] = 1.0

    in_maps = []
    for k in range(N_CORES):
        nv_k = np.roll(nv_ext, -k * P, axis=1)
        ch_k = colhs.copy()
        for b in range(rpc):
            cols = slice(b * 8 * P, b * 8 * P + P)
            dst = slice(641 + b * P, 641 + (b + 1) * P)
            ch_k[:C, dst] = nv_k[:C, cols]
            ch_k[C, dst] = 1.0
            ch_k[C + 1, dst] = nv_k[C, cols]  # -1e9*padrow
        in_maps.append({
            "nv": nv_k.astype(bf16),
            "colhs": ch_k.astype(bf16),
        })

    nc = _get_kernel(NBp)
    res = run_bass_kernel_spmd(nc, in_maps, core_ids=list(range(N_CORES)),
                               trace=_trace)
    _results[0] = res

    # host epilogue: map class-pure partials into S[class, col]
    S = np.zeros((NUM_CLASSES, Mp), np.float64)
    for k in range(N_CORES):
        rs = np.asarray(res.results[k]["rs_out"], np.float64)
        cs = np.asarray(res.results[k]["cs_out"], np.float64).reshape(-1)
        for b in range(rpc):
            r = (k + 8 * b) % NBp
            rows_glob = (np.arange(P) + 8 * b * P + k * P) % Mp
            for w in range(W):
                cls = block_class[(r + w) % NBp]
                if cls >= 0:
                    S[cls, rows_glob] += rs[:, b * W + w]
            cls_r = block_class[r]
            if cls_r >= 0:
                for w in range(1, W - 1):
                    c = (r + w) % NBp
                    seg = cs[b * CSW + (w - 1) * P: b * CSW + w * P]
                    S[cls_r, c * P:(c + 1) * P] += seg

    pos = S[lab_sorted, col_of_point]
    denom = S[:, col_of_point].sum(axis=0)
    per = -np.log(pos / denom)
    loss = np.float32(per.sum() / np.float64(n_div))
    return np.asarray(loss, dtype=np.float32)


# revision 16
# speedup vs baseline: 19.2568x; 19.2568x over previous
"""Trainium2 Bass kernel for the intra-batch point-cloud contrastive loss.

Math (matches the reference exactly):
  feats   = features_in.reshape(C, M).T    (row-major reinterpret), M = B*N
  labels  = labels_in.reshape(-1)
  sel     = bernoulli(key 42, min(750/(count+1),1)[labels])   (host, jax CPU)
  nv      = feats / ||feats||
  dp      = exp(nv @ nv.T / TEMP), diagonal zeroed
  pos_i   = sum_{j sel, same class} dp_ij ; neg over different class
  loss    = mean over selected i of -log(pos/(pos+neg))

Strategy (triangle/circulant, exp-minimal):
  Only selected points matter (~37% of M).  The selected points are
  class-sorted and each class padded to a 128-col boundary, so every
  128-block of columns is class-pure.  dp is symmetric, so each
  unordered block-pair is computed ONCE: row-block r covers col-blocks
  c = r+d (mod NB) for d in 0..NB/2.  Each core gets NB/8 row-blocks
  (circulant: core k takes global blocks {k, k+8, ...}; its input is
  rolled by k*128 so all slice offsets are SPMD-uniform).

  Per row-block window [128, W*128] (W = NB/2+1 blocks):
    PE  : sim chunks = lhsT.T @ nv  (bf16, K=66: 64 feature rows + a
          "-1e9 * pad" row pair masking pad rows/cols pre-exp), plus an
          eye@negeye matmul adding -1e9 on the d=0 diagonal
    ACT : dp = exp(sim/TEMP) -> bf16 SBUF, one instruction per window
          (exp volume is the throughput wall; the triangle halves it)
    DVE : per-128-block row sums (one 3D-AP tensor_reduce per window)
          -> class-pure partials, class-mapped on the host
    Pool: column sums of blocks d=1..W-2 via partition_all_reduce (the
          transpose-side contribution; d=0/W-1 excluded to avoid double
          counting)
  The host maps partials to classes and runs the O(n_sel) epilogue.
  Instruction count is kept minimal (~30 PE instrs): the fixed
  ~57-semaphore teardown sweep runs at ~280 ns/op on the PE sequencer,
  so a long PE instruction stream directly extends exec time.
"""

import numpy as np

TEMP = 0.07
NUM_CLASSES = 4
N_CORES = 8
P = 128

_NEFF_CACHE = {}


def _compute_sel(labels_flat):
    """Selection mask, bit-exact with the reference (jax threefry, key 42)."""
    import jax
    import jax.numpy as jnp

    cpu = jax.devices("cpu")[0]
    with jax.default_device(cpu):
        lab_j = jnp.asarray(labels_flat)
        counts = jnp.bincount(lab_j, length=NUM_CLASSES)
        keep_p = jnp.minimum(750.0 / (counts.astype(jnp.float32) + 1.0), 1.0)
        p = keep_p[lab_j]
        sel = jax.random.bernoulli(jax.random.key(42), p)
        return np.asarray(sel)


def _build_kernel(NBp):
    import concourse.bass as bass
    import concourse.bass_isa as bass_isa
    import concourse.mybir as mybir
    import concourse.tile as tile

    Mp = NBp * P
    rpc = NBp // N_CORES                  # row-blocks per core
    W = NBp // 2 + 1                      # window width (blocks)
    WP = W * P
    CSW = (W - 2) * P                     # colsum width per row-block
    f32 = mybir.dt.float32
    bf16 = mybir.dt.bfloat16
    K = 66                                # 64 features + colmask + rowmask

    nc = bass.Bass()
    nv_d = nc.dram_tensor("nv", [K, Mp], bf16, kind="ExternalInput")
    # colhs: eye [0:128], negeye-wide [128:640], ones col [640:641],
    # lhs blocks [641 : 641+rpc*128] (partitions 0..65)
    colhs_d = nc.dram_tensor("colhs", [P, 641 + rpc * P], bf16,
                             kind="ExternalInput")
    rs_d = nc.dram_tensor("rs_out", [P, rpc * W], f32, kind="ExternalOutput")
    cs_d = nc.dram_tensor("cs_out", [P, rpc * (W - 2)], f32,
                          kind="ExternalOutput")

    AW = min(1024, WP)                    # chunk A cols, B = rest
    BW = WP - AW
    with tile.TileContext(nc) as tc:
        with (
            tc.tile_pool(name="singles", bufs=1) as singles,
            tc.tile_pool(name="dp", bufs=2) as dp_pool,
            tc.tile_pool(name="psa", bufs=2, space="PSUM") as psa_pool,
            tc.tile_pool(name="psb", bufs=1, space="PSUM") as psb_pool,
            tc.tile_pool(name="cs", bufs=1, space="PSUM") as cs_pool,
        ):
            colhs_sb = singles.tile([P, 641 + rpc * P], bf16)
            nv_sb = singles.tile([K, Mp], bf16)
            nc.gpsimd.dma_start(out=colhs_sb[:], in_=colhs_d[:])
            # first split covers row-block 0's whole window
            cut = min(WP, Mp)
            nc.gpsimd.dma_start(out=nv_sb[:, 0:cut], in_=nv_d[:, 0:cut])
            if cut < Mp:
                nc.gpsimd.dma_start(out=nv_sb[:, cut:Mp], in_=nv_d[:, cut:Mp])

            eye = colhs_sb[:, 0:P]
            negw = colhs_sb[:, P:P + 512]
            ones_col = colhs_sb[:, 640:641]
            lhs_sb = colhs_sb[0:K, 641:641 + rpc * P]

            rs_sb = singles.tile([P, rpc * W], f32)
            cs_sb = singles.tile([P, rpc * (W - 2)], f32)
            cs_ps = cs_pool.tile([P, rpc * (W - 2)], f32)

            for b in range(rpc):
                lhs_b = lhs_sb[:, b * P:(b + 1) * P]
                dp = dp_pool.tile([P, WP], bf16)
                for lo, wid, pool in ((0, AW, psa_pool), (AW, BW, psb_pool)):
                    if wid == 0:
                        continue
                    ps = pool.tile([P, wid], f32, name=f"ps{lo != 0:d}")
                    off = 0
                    while off < wid:
                        w512 = min(512, wid - off)
                        # rolled col of this piece (wraps at 1024 bounds)
                        src = (b * 1024 + lo + off) % Mp
                        nc.tensor.matmul(
                            ps[:, off:off + w512],
                            lhs_b,
                            nv_sb[:, src:src + w512],
                            start=True,
                            stop=(lo + off) != 0,
                        )
                        if lo + off == 0:
                            # adds -1e9 on the diagonal of window block 0
                            nc.tensor.matmul(
                                ps[:, 0:w512], eye, negw[:, 0:w512],
                                start=False, stop=True,
                            )
                        off += w512
                    nc.scalar.activation(
                        dp[:, lo:lo + wid], ps[:, 0:wid],
                        mybir.ActivationFunctionType.Exp,
                        scale=float(1.0 / TEMP),
                    )
                    # column sums of window blocks [max(1,lo/P), min(W-1,
                    # (lo+wid)/P)): dp block as stationary weights, ones as
                    # the 1-col moving operand -> [128, 1] psum each
                    for w in range(max(1, lo // P), min(W - 1, (lo + wid) // P)):
                        nc.tensor.matmul(
                            cs_ps[:, b * (W - 2) + w - 1:b * (W - 2) + w],
                            dp[:, w * P:(w + 1) * P],
                            ones_col,
                            start=True, stop=True,
                        )
                # per-block row sums (class-pure partials)
                nc.vector.tensor_reduce(
                    rs_sb[:, b * W:(b + 1) * W],
                    dp[:].rearrange("p (a b) -> p a b", b=P),
                    mybir.AxisListType.X,
                    mybir.AluOpType.add,
                )
            nc.vector.tensor_scalar_add(cs_sb[:], cs_ps[:], 0.0)
            nc.gpsimd.dma_start(out=cs_d[:], in_=cs_sb[:])
            nc.gpsimd.dma_start(out=rs_d[:], in_=rs_sb[:])

    _split_multi_waits(nc)
    return nc


def _split_multi_waits(nc):
    """Walrus in this toolchain accepts only one inline sync-wait per
    instruction.  Tile's kernel-tail drain aggregates one wait per live
    semaphore, so hoist all but the last wait onto nops.

    Drain waits go to SP-sequencer nops: the PE sequencer retires a
    semaphore op in ~280 ns vs ~25 ns on SP, so leaving the aggregated
    end-of-kernel waits on PE adds ~5 us of pure teardown.  Mid-kernel
    (non-drain) waits must stay on the owning engine to preserve
    ordering."""
    import concourse.mybir as mybir

    for fn in nc.m.functions:
        for blk in fn.blocks:
            insts = list(blk.instructions)
            out = []
            for inst in insts:
                si = inst.sync_info
                waits = list(si.on_wait) if si is not None and si.on_wait else []
                is_drain = isinstance(inst, mybir.InstDrain)
                if is_drain and waits:
                    for w in waits:
                        out.append(mybir.InstNoOp(
                            name=nc.get_next_instruction_name(),
                            engine=mybir.EngineType.SP,
                            bass_nofuse=True,
                            sync_info=mybir.SyncInfo(on_wait=[w], on_update=[]),
                        ))
                    si.on_wait = []
                elif len(waits) > 1:
                    for w in waits[:-1]:
                        out.append(mybir.InstNoOp(
                            name=nc.get_next_instruction_name(),
                            engine=inst.engine,
                            bass_nofuse=True,
                            sync_info=mybir.SyncInfo(on_wait=[w], on_update=[]),
                        ))
                    si.on_wait = waits[-1:]
                out.append(inst)
            if len(out) != len(insts):
                blk.instructions = out
    return nc


def _get_kernel(NBp):
    if NBp not in _NEFF_CACHE:
        _NEFF_CACHE[NBp] = _build_kernel(NBp)
    return _NEFF_CACHE[NBp]


def kernel(features_in, labels_in, _trace=False, _results=[None]):
    import ml_dtypes
    from concourse.bass_utils import run_bass_kernel_spmd

    bf16 = ml_dtypes.bfloat16
    features_in = np.asarray(features_in, dtype=np.float32)
    B, C, N = features_in.shape
    M = B * N
    labels = np.asarray(labels_in).reshape(-1).astype(np.int64)

    fT = features_in.reshape(C, M)                      # [C, M] reinterpret
    sel = _compute_sel(labels)
    idx = np.nonzero(sel)[0]
    lab_sel = labels[idx]
    n_sel = int(idx.size)
    n_div = max(n_sel, 1)

    # class-sorted, per-class 128-padded column layout
    order = np.argsort(lab_sel, kind="stable")
    idx_sorted = idx[order]
    lab_sorted = lab_sel[order]
    cnt = np.bincount(lab_sel, minlength=NUM_CLASSES)
    cls_blocks = np.maximum(np.ceil(cnt / P).astype(int), cnt > 0)
    NB = max(int(cls_blocks.sum()), 1)
    NBp = max(8 * int(np.ceil(NB / 8)), 8)
    Mp = NBp * P
    rpc = NBp // N_CORES
    W = NBp // 2 + 1
    CSW = (W - 2) * P

    norms = np.sqrt(np.sum(fT * fT, axis=0, dtype=np.float32))
    nvT = (fT / norms).astype(np.float32)

    col_of_point = np.zeros(n_sel, np.int64)
    block_class = np.full(NBp, -1, np.int64)
    nv = np.zeros((C, Mp), np.float32)
    padcol = np.ones(Mp, bool)
    b0 = 0
    pos_pt = 0
    for c in range(NUM_CLASSES):
        start = b0 * P
        n = int(cnt[c])
        sl = slice(pos_pt, pos_pt + n)
        col_of_point[sl] = start + np.arange(n)
        nv[:, start:start + n] = nvT[:, idx_sorted[sl]]
        padcol[start:start + n] = False
        block_class[b0:b0 + int(cls_blocks[c])] = c
        b0 += int(cls_blocks[c])
        pos_pt += n

    K = 66
    nv_ext = np.zeros((K, Mp), np.float32)
    nv_ext[:C] = nv
    nv_ext[C] = -1e9 * padcol                 # colmask row
    nv_ext[C + 1] = 1.0                       # pairs with lhs rowmask row

    colhs = np.zeros((P, 641 + rpc * P), np.float32)
    colhs[:, 0:P] = np.eye(P, dtype=np.float32)
    colhs[np.arange(P), P + np.arange(P)] = -1e9   # negeye (wide, zero-padded)
    colhs[:, 640] = 1.0

    in_maps = []
    for k in range(N_CORES):
        nv_k = np.roll(nv_ext, -k * P, axis=1)
        ch_k = colhs.copy()
        for b in range(rpc):
            cols = slice(b * 8 * P, b * 8 * P + P)
            dst = slice(641 + b * P, 641 + (b + 1) * P)
            ch_k[:C, dst] = nv_k[:C, cols]
            ch_k[C, dst] = 1.0
            ch_k[C + 1, dst] = nv_k[C, cols]  # -1e9*padrow
        in_maps.append({
            "nv": nv_k.astype(bf16),
            "colhs": ch_k.astype(bf16),
        })

    nc = _get_kernel(NBp)
    res = run_bass_kernel_spmd(nc, in_maps, core_ids=list(range(N_CORES)),
                               trace=_trace)
    _results[0] = res

    # host epilogue: map class-pure partials into S[class, col]
    S = np.zeros((NUM_CLASSES, Mp), np.float64)
    for k in range(N_CORES):
        rs = np.asarray(res.results[k]["rs_out"], np.float64)
        cs = np.asarray(res.results[k]["cs_out"], np.float64)
        for b in range(rpc):
            r = (k + 8 * b) % NBp
            rows_glob = (np.arange(P) + 8 * b * P + k * P) % Mp
            for w in range(W):
                cls = block_class[(r + w) % NBp]
                if cls >= 0:
                    S[cls, rows_glob] += rs[:, b * W + w]
            cls_r = block_class[r]
            if cls_r >= 0:
                for w in range(1, W - 1):
                    c = (r + w) % NBp
                    S[cls_r, c * P:(c + 1) * P] += cs[:, b * (W - 2) + w - 1]

    pos = S[lab_sorted, col_of_point]
    denom = S[:, col_of_point].sum(axis=0)
    per = -np.log(pos / denom)
    loss = np.float32(per.sum() / np.float64(n_div))
    return np.asarray(loss, dtype=np.float32)
